# Initial kernel scaffold
#
"""Bilinear RoI pooling (7x7) on 8 Trainium2 NeuronCores.

Strategy (data-parallel over RoIs, per the sharding hint):
  - B=1024 boxes split into 8 slices of 128; the feature map is replicated.
  - Host pads features (128,128,512) -> (130,132,512) with a zero border
    (1 row/col on top/left is enough for the "-1" corner, 2 pad cols on the
    left and right make the x0/x0+1 pair-gather safe for far out-of-bounds
    coords). Out-of-bounds bilinear corners then read zero rows instead of
    needing an in-bounds mask: indices are clamped so that any corner the
    reference would mask out lands on a zero row/col of the padded map.
  - Per core, per (box, grid-point): two indirect-DMA gathers of 1024
    contiguous f32 (the x0/x0+1 row pair for each of the two y corners),
    then a 4-term per-partition weighted sum on the vector engine.

Device layout: one point-tile = one of the 49 grid positions across all
128 boxes (partition = box). 49 iterations of gather+blend+store.
"""

import numpy as np

P = 128          # boxes per core == SBUF partitions
C = 512          # channels
NPT = 49         # 7*7 grid points
HP, WP = 130, 132
NROW = HP * WP   # 17160 padded rows of C floats
NCORES = 8
MAGIC = 12582912.0  # 1.5*2^23: x+MAGIC stays in [2^23,2^24) where ulp == 1

_STATE = {}


def _build_nc():
    import concourse.bass as bass
    import concourse.tile as tile
    from concourse import mybir

    F32 = mybir.dt.float32
    I32 = mybir.dt.int32
    Alu = mybir.AluOpType

    nc = bass.Bass()
    fpad = nc.declare_dram_parameter("fpad", [NROW, C], F32, isOutput=False)
    boxes = nc.declare_dram_parameter("boxes", [P, 4], F32, isOutput=False)
    grid = nc.declare_dram_parameter("grid", [P, 2 * NPT], F32, isOutput=False)
    out = nc.declare_dram_parameter("out", [P, NPT * C], F32, isOutput=True)

    with tile.TileContext(nc) as tc:
        with (
            tc.tile_pool(name="const", bufs=1) as cpool,
            tc.tile_pool(name="work", bufs=8) as wpool,
        ):
            bx = cpool.tile([P, 4], F32)
            nc.sync.dma_start(out=bx[:], in_=boxes[:])
            g = cpool.tile([P, 2 * NPT], F32)
            nc.sync.dma_start(out=g[:], in_=grid[:])
            BY = g[:, 0:NPT]
            BX = g[:, NPT:2 * NPT]

            xc, yc = bx[:, 0:1], bx[:, 1:2]
            bw, bh = bx[:, 2:3], bx[:, 3:4]

            # per-box scale/translate: yf = BY*(0.5*bh-0.5) + (yc-1)
            sy = cpool.tile([P, 1], F32)
            nc.vector.tensor_scalar(out=sy[:], in0=bh, scalar1=0.5, scalar2=-0.5,
                                    op0=Alu.mult, op1=Alu.add)
            sx = cpool.tile([P, 1], F32)
            nc.vector.tensor_scalar(out=sx[:], in0=bw, scalar1=0.5, scalar2=-0.5,
                                    op0=Alu.mult, op1=Alu.add)
            ty = cpool.tile([P, 1], F32)
            nc.vector.tensor_scalar(out=ty[:], in0=yc, scalar1=-1.0, scalar2=None,
                                    op0=Alu.add)
            tx = cpool.tile([P, 1], F32)
            nc.vector.tensor_scalar(out=tx[:], in0=xc, scalar1=-1.0, scalar2=None,
                                    op0=Alu.add)

            yf = cpool.tile([P, NPT], F32)
            nc.vector.tensor_scalar(out=yf[:], in0=BY, scalar1=sy[:, 0:1],
                                    scalar2=ty[:, 0:1], op0=Alu.mult, op1=Alu.add)
            xf = cpool.tile([P, NPT], F32)
            nc.vector.tensor_scalar(out=xf[:], in0=BX, scalar1=sx[:, 0:1],
                                    scalar2=tx[:, 0:1], op0=Alu.mult, op1=Alu.add)

            def floor_frac(src):
                r = cpool.tile([P, NPT], F32, tag=f"r{src.tensor.name}")
                nc.vector.tensor_scalar(out=r[:], in0=src[:], scalar1=MAGIC,
                                        scalar2=-MAGIC, op0=Alu.add, op1=Alu.add)
                m = cpool.tile([P, NPT], F32, tag=f"m{src.tensor.name}")
                nc.vector.tensor_tensor(out=m[:], in0=r[:], in1=src[:], op=Alu.is_gt)
                fl = cpool.tile([P, NPT], F32, tag=f"f{src.tensor.name}")
                nc.vector.tensor_tensor(out=fl[:], in0=r[:], in1=m[:], op=Alu.subtract)
                fr = cpool.tile([P, NPT], F32, tag=f"w{src.tensor.name}")
                nc.vector.tensor_tensor(out=fr[:], in0=src[:], in1=fl[:], op=Alu.subtract)
                return fl, fr

            y0, wy = floor_frac(yf[:])
            x0, wx = floor_frac(xf[:])

            wyc = cpool.tile([P, NPT], F32)
            nc.vector.tensor_scalar(out=wyc[:], in0=wy[:], scalar1=-1.0, scalar2=1.0,
                                    op0=Alu.mult, op1=Alu.add)
            wxc = cpool.tile([P, NPT], F32)
            nc.vector.tensor_scalar(out=wxc[:], in0=wx[:], scalar1=-1.0, scalar2=1.0,
                                    op0=Alu.mult, op1=Alu.add)

            wA0 = cpool.tile([P, NPT], F32)
            nc.vector.tensor_tensor(out=wA0[:], in0=wyc[:], in1=wxc[:], op=Alu.mult)
            wA1 = cpool.tile([P, NPT], F32)
            nc.vector.tensor_tensor(out=wA1[:], in0=wyc[:], in1=wx[:], op=Alu.mult)
            wB0 = cpool.tile([P, NPT], F32)
            nc.vector.tensor_tensor(out=wB0[:], in0=wy[:], in1=wxc[:], op=Alu.mult)
            wB1 = cpool.tile([P, NPT], F32)
            nc.vector.tensor_tensor(out=wB1[:], in0=wy[:], in1=wx[:], op=Alu.mult)

            # gather row indices into the padded (130,132) map, in 512-elem units:
            #   idxA = (clamp(y0,-1,128)+1)*132 + clamp(x0,-2,128)+2
            #   idxB = (clamp(y0,-2,127)+2)*132 + clamp(x0,-2,128)+2
            cy0 = cpool.tile([P, NPT], F32)
            nc.vector.tensor_scalar(out=cy0[:], in0=y0[:], scalar1=-1.0, scalar2=128.0,
                                    op0=Alu.max, op1=Alu.min)
            cy1 = cpool.tile([P, NPT], F32)
            nc.vector.tensor_scalar(out=cy1[:], in0=y0[:], scalar1=127.0, scalar2=-2.0,
                                    op0=Alu.min, op1=Alu.max)
            cxc = cpool.tile([P, NPT], F32)
            nc.vector.tensor_scalar(out=cxc[:], in0=x0[:], scalar1=-2.0, scalar2=128.0,
                                    op0=Alu.max, op1=Alu.min)

            idxA = cpool.tile([P, NPT], I32)
            affA = cpool.tile([P, NPT], F32)
            nc.vector.tensor_scalar(out=affA[:], in0=cy0[:], scalar1=float(WP),
                                    scalar2=float(WP + 2), op0=Alu.mult, op1=Alu.add)
            nc.vector.tensor_tensor(out=affA[:], in0=affA[:], in1=cxc[:], op=Alu.add)
            nc.vector.tensor_copy(out=idxA[:], in_=affA[:])

            idxB = cpool.tile([P, NPT], I32)
            affB = cpool.tile([P, NPT], F32)
            nc.vector.tensor_scalar(out=affB[:], in0=cy1[:], scalar1=float(WP),
                                    scalar2=float(2 * WP + 2), op0=Alu.mult, op1=Alu.add)
            nc.vector.tensor_tensor(out=affB[:], in0=affB[:], in1=cxc[:], op=Alu.add)
            nc.vector.tensor_copy(out=idxB[:], in_=affB[:])

            import concourse.bass as _b
            for t in range(NPT):
                gA = wpool.tile([P, 2 * C], F32, tag="gA")
                nc.gpsimd.indirect_dma_start(
                    out=gA[:], out_offset=None, in_=fpad[:],
                    in_offset=_b.IndirectOffsetOnAxis(ap=idxA[:, t:t + 1], axis=0),
                )
                gB = wpool.tile([P, 2 * C], F32, tag="gB")
                nc.gpsimd.indirect_dma_start(
                    out=gB[:], out_offset=None, in_=fpad[:],
                    in_offset=_b.IndirectOffsetOnAxis(ap=idxB[:, t:t + 1], axis=0),
                )
                acc = wpool.tile([P, C], F32, tag="acc")
                nc.vector.tensor_scalar(out=acc[:], in0=gA[:, 0:C],
                                        scalar1=wA0[:, t:t + 1], scalar2=None,
                                        op0=Alu.mult)
                nc.vector.scalar_tensor_tensor(out=acc[:], in0=gA[:, C:2 * C],
                                               scalar=wA1[:, t:t + 1], in1=acc[:],
                                               op0=Alu.mult, op1=Alu.add)
                nc.vector.scalar_tensor_tensor(out=acc[:], in0=gB[:, 0:C],
                                               scalar=wB0[:, t:t + 1], in1=acc[:],
                                               op0=Alu.mult, op1=Alu.add)
                nc.vector.scalar_tensor_tensor(out=acc[:], in0=gB[:, C:2 * C],
                                               scalar=wB1[:, t:t + 1], in1=acc[:],
                                               op0=Alu.mult, op1=Alu.add)
                nc.sync.dma_start(out=out[:, t * C:(t + 1) * C], in_=acc[:])

    return nc


def _grid_const():
    base = np.linspace(-1.0, 1.0, 7).astype(np.float32)
    by = np.repeat(base, 7)
    bxx = np.tile(base, 7)
    g = np.concatenate([by, bxx])[None, :]
    return np.ascontiguousarray(np.broadcast_to(g, (P, 2 * NPT)).astype(np.float32))


def _pad_features(features):
    fp = np.zeros((HP, WP, C), dtype=np.float32)
    fp[1:129, 2:130, :] = features
    return fp.reshape(NROW, C)


def kernel(features, boxes, image_height=128, image_width=128):
    from concourse.bass_utils import run_bass_kernel_spmd

    if "nc" not in _STATE:
        _STATE["nc"] = _build_nc()
        _STATE["grid"] = _grid_const()
    nc = _STATE["nc"]

    fpad = _pad_features(np.asarray(features, dtype=np.float32))
    boxes = np.asarray(boxes, dtype=np.float32)
    gridc = _STATE["grid"]
    in_maps = [
        {
            "fpad": fpad,
            "boxes": np.ascontiguousarray(boxes[k * P:(k + 1) * P]),
            "grid": gridc,
        }
        for k in range(NCORES)
    ]
    res = run_bass_kernel_spmd(nc, in_maps, core_ids=list(range(NCORES)))
    out = np.concatenate(
        [res.results[k]["out"].reshape(P, 7, 7, C) for k in range(NCORES)], axis=0
    )
    return out


# revision 9
# speedup vs baseline: 1.1369x; 1.1369x over previous
"""Bilinear RoI pooling (7x7) on 8 Trainium2 NeuronCores.

Strategy (data-parallel over RoIs, per the sharding hint):
  - B=1024 boxes split into 8 slices of 128; the feature map is replicated.
  - Host pads features (128,128,512) -> (130,132,512) with a zero border
    (1 row/col on top/left is enough for the "-1" corner, 2 pad cols on the
    left and right make the x0/x0+1 pair-gather safe for far out-of-bounds
    coords). Out-of-bounds bilinear corners then read zero rows instead of
    needing an in-bounds mask: indices are clamped so that any corner the
    reference would mask out lands on a zero row/col of the padded map.
  - Per core, per (box, grid-point): two indirect-DMA gathers of 1024
    contiguous f32 (the x0/x0+1 row pair for each of the two y corners),
    then a 4-term per-partition weighted sum on the vector engine.

Device layout: one point-tile = one of the 49 grid positions across all
128 boxes (partition = box). 49 iterations of gather+blend+store.
"""

import numpy as np

P = 128          # boxes per core == SBUF partitions
C = 512          # channels
NPT = 49         # 7*7 grid points
HP, WP = 130, 132
NROW = HP * WP   # 17160 padded rows of C floats
NCORES = 8
MAGIC = 12582912.0  # 1.5*2^23: x+MAGIC stays in [2^23,2^24) where ulp == 1

_STATE = {}


def _build_nc(repeats=1):
    import concourse.bass as bass
    import concourse.bacc as bacc
    import concourse.tile as tile
    from concourse import mybir

    F32 = mybir.dt.float32
    I32 = mybir.dt.int32
    Alu = mybir.AluOpType

    nc = bacc.Bacc()
    fpad = nc.declare_dram_parameter("fpad", [NROW, C], F32, isOutput=False)
    boxes = nc.declare_dram_parameter("boxes", [P, 4], F32, isOutput=False)
    grid = nc.declare_dram_parameter("grid", [P, 2 * NPT], F32, isOutput=False)
    out = nc.declare_dram_parameter("out", [P, NPT * C], F32, isOutput=True)

    with tile.TileContext(nc) as tc:
        with (
            tc.tile_pool(name="const", bufs=1) as cpool,
            tc.tile_pool(name="work", bufs=8) as wpool,
        ):
            bx_in = cpool.tile([P, 4], F32)
            nc.sync.dma_start(out=bx_in[:], in_=boxes[:])
            g_in = cpool.tile([P, 2 * NPT], F32)
            nc.sync.dma_start(out=g_in[:], in_=grid[:])
            # plain copies absorb the DMA sem waits: TensorScalarPtr-family
            # ops downstream have too few ISA sync slots for >1 wait
            bx = cpool.tile([P, 4], F32)
            nc.vector.tensor_copy(out=bx[:], in_=bx_in[:])
            g = cpool.tile([P, 2 * NPT], F32)
            nc.vector.tensor_copy(out=g[:], in_=g_in[:])
            BY = g[:, 0:NPT]
            BX = g[:, NPT:2 * NPT]

            xc, yc = bx[:, 0:1], bx[:, 1:2]
            bw, bh = bx[:, 2:3], bx[:, 3:4]

            # per-box scale/translate: yf = BY*(0.5*bh-0.5) + (yc-1)
            sy = cpool.tile([P, 1], F32)
            nc.vector.tensor_scalar(out=sy[:], in0=bh, scalar1=0.5, scalar2=-0.5,
                                    op0=Alu.mult, op1=Alu.add)
            sx = cpool.tile([P, 1], F32)
            nc.vector.tensor_scalar(out=sx[:], in0=bw, scalar1=0.5, scalar2=-0.5,
                                    op0=Alu.mult, op1=Alu.add)
            ty = cpool.tile([P, 1], F32)
            nc.vector.tensor_scalar(out=ty[:], in0=yc, scalar1=-1.0, scalar2=None,
                                    op0=Alu.add)
            tx = cpool.tile([P, 1], F32)
            nc.vector.tensor_scalar(out=tx[:], in0=xc, scalar1=-1.0, scalar2=None,
                                    op0=Alu.add)

            yf = cpool.tile([P, NPT], F32)
            nc.vector.tensor_scalar(out=yf[:], in0=BY, scalar1=sy[:, 0:1],
                                    scalar2=ty[:, 0:1], op0=Alu.mult, op1=Alu.add)
            xf = cpool.tile([P, NPT], F32)
            nc.vector.tensor_scalar(out=xf[:], in0=BX, scalar1=sx[:, 0:1],
                                    scalar2=tx[:, 0:1], op0=Alu.mult, op1=Alu.add)

            def floor_frac(src):
                r = cpool.tile([P, NPT], F32, tag=f"r{src.tensor.name}")
                nc.vector.tensor_scalar(out=r[:], in0=src[:], scalar1=MAGIC,
                                        scalar2=-MAGIC, op0=Alu.add, op1=Alu.add)
                m = cpool.tile([P, NPT], F32, tag=f"m{src.tensor.name}")
                nc.vector.tensor_tensor(out=m[:], in0=r[:], in1=src[:], op=Alu.is_gt)
                fl = cpool.tile([P, NPT], F32, tag=f"f{src.tensor.name}")
                nc.vector.tensor_tensor(out=fl[:], in0=r[:], in1=m[:], op=Alu.subtract)
                fr = cpool.tile([P, NPT], F32, tag=f"w{src.tensor.name}")
                nc.vector.tensor_tensor(out=fr[:], in0=src[:], in1=fl[:], op=Alu.subtract)
                return fl, fr

            y0, wy = floor_frac(yf[:])
            x0, wx = floor_frac(xf[:])

            wyc = cpool.tile([P, NPT], F32)
            nc.vector.tensor_scalar(out=wyc[:], in0=wy[:], scalar1=-1.0, scalar2=1.0,
                                    op0=Alu.mult, op1=Alu.add)
            wxc = cpool.tile([P, NPT], F32)
            nc.vector.tensor_scalar(out=wxc[:], in0=wx[:], scalar1=-1.0, scalar2=1.0,
                                    op0=Alu.mult, op1=Alu.add)

            wA0 = cpool.tile([P, NPT], F32)
            nc.vector.tensor_tensor(out=wA0[:], in0=wyc[:], in1=wxc[:], op=Alu.mult)
            wA1 = cpool.tile([P, NPT], F32)
            nc.vector.tensor_tensor(out=wA1[:], in0=wyc[:], in1=wx[:], op=Alu.mult)
            wB0 = cpool.tile([P, NPT], F32)
            nc.vector.tensor_tensor(out=wB0[:], in0=wy[:], in1=wxc[:], op=Alu.mult)
            wB1 = cpool.tile([P, NPT], F32)
            nc.vector.tensor_tensor(out=wB1[:], in0=wy[:], in1=wx[:], op=Alu.mult)

            # gather row indices into the padded (130,132) map, in 512-elem units:
            #   idxA = (clamp(y0,-1,128)+1)*132 + clamp(x0,-2,128)+2
            #   idxB = (clamp(y0,-2,127)+2)*132 + clamp(x0,-2,128)+2
            cy0 = cpool.tile([P, NPT], F32)
            nc.vector.tensor_scalar(out=cy0[:], in0=y0[:], scalar1=-1.0, scalar2=128.0,
                                    op0=Alu.max, op1=Alu.min)
            cy1 = cpool.tile([P, NPT], F32)
            nc.vector.tensor_scalar(out=cy1[:], in0=y0[:], scalar1=127.0, scalar2=-2.0,
                                    op0=Alu.min, op1=Alu.max)
            cxc = cpool.tile([P, NPT], F32)
            nc.vector.tensor_scalar(out=cxc[:], in0=x0[:], scalar1=-2.0, scalar2=128.0,
                                    op0=Alu.max, op1=Alu.min)

            idxA = cpool.tile([P, NPT], I32)
            affA = cpool.tile([P, NPT], F32)
            nc.vector.tensor_scalar(out=affA[:], in0=cy0[:], scalar1=float(WP),
                                    scalar2=float(WP + 2), op0=Alu.mult, op1=Alu.add)
            nc.vector.tensor_tensor(out=affA[:], in0=affA[:], in1=cxc[:], op=Alu.add)
            nc.vector.tensor_copy(out=idxA[:], in_=affA[:])

            idxB = cpool.tile([P, NPT], I32)
            affB = cpool.tile([P, NPT], F32)
            nc.vector.tensor_scalar(out=affB[:], in0=cy1[:], scalar1=float(WP),
                                    scalar2=float(2 * WP + 2), op0=Alu.mult, op1=Alu.add)
            nc.vector.tensor_tensor(out=affB[:], in0=affB[:], in1=cxc[:], op=Alu.add)
            nc.vector.tensor_copy(out=idxB[:], in_=affB[:])

            import concourse.bass as _b
            for t in [t for _ in range(repeats) for t in range(NPT)]:
                gA = wpool.tile([P, 2 * C], F32, tag="gA")
                nc.gpsimd.indirect_dma_start(
                    out=gA[:], out_offset=None, in_=fpad[:],
                    in_offset=_b.IndirectOffsetOnAxis(ap=idxA[:, t:t + 1], axis=0),
                )
                gB = wpool.tile([P, 2 * C], F32, tag="gB")
                nc.gpsimd.indirect_dma_start(
                    out=gB[:], out_offset=None, in_=fpad[:],
                    in_offset=_b.IndirectOffsetOnAxis(ap=idxB[:, t:t + 1], axis=0),
                )
                acc = wpool.tile([P, C], F32, tag="acc")
                nc.vector.tensor_scalar(out=acc[:], in0=gA[:, 0:C],
                                        scalar1=wA0[:, t:t + 1], scalar2=None,
                                        op0=Alu.mult)
                nc.vector.scalar_tensor_tensor(out=acc[:], in0=gA[:, C:2 * C],
                                               scalar=wA1[:, t:t + 1], in1=acc[:],
                                               op0=Alu.mult, op1=Alu.add)
                nc.vector.scalar_tensor_tensor(out=acc[:], in0=gB[:, 0:C],
                                               scalar=wB0[:, t:t + 1], in1=acc[:],
                                               op0=Alu.mult, op1=Alu.add)
                nc.vector.scalar_tensor_tensor(out=acc[:], in0=gB[:, C:2 * C],
                                               scalar=wB1[:, t:t + 1], in1=acc[:],
                                               op0=Alu.mult, op1=Alu.add)
                nc.sync.dma_start(out=out[:, t * C:(t + 1) * C], in_=acc[:])

    nc.compile()
    return nc


def _grid_const():
    base = np.linspace(-1.0, 1.0, 7).astype(np.float32)
    by = np.repeat(base, 7)
    bxx = np.tile(base, 7)
    g = np.concatenate([by, bxx])[None, :]
    return np.ascontiguousarray(np.broadcast_to(g, (P, 2 * NPT)).astype(np.float32))


def _pad_features(features):
    fp = np.zeros((HP, WP, C), dtype=np.float32)
    fp[1:129, 2:130, :] = features
    return fp.reshape(NROW, C)


def kernel(features, boxes, image_height=128, image_width=128):
    from concourse.bass_utils import run_bass_kernel_spmd

    if "nc" not in _STATE:
        _STATE["nc"] = _build_nc()
        _STATE["grid"] = _grid_const()
    nc = _STATE["nc"]

    fpad = _pad_features(np.asarray(features, dtype=np.float32))
    boxes = np.asarray(boxes, dtype=np.float32)
    gridc = _STATE["grid"]
    in_maps = [
        {
            "fpad": fpad,
            "boxes": np.ascontiguousarray(boxes[k * P:(k + 1) * P]),
            "grid": gridc,
        }
        for k in range(NCORES)
    ]
    res = run_bass_kernel_spmd(
        nc, in_maps, core_ids=list(range(NCORES)),
        trace=_STATE.get("trace", False),
    )
    _STATE["last"] = res
    out = np.concatenate(
        [res.results[k]["out"].reshape(P, 7, 7, C) for k in range(NCORES)], axis=0
    )
    return out
